# revision 1
# baseline (speedup 1.0000x reference)
"""ContactMapHead Trainium2 kernel.

Reference computation (per batch b):
    h = relu(X @ W^T + pb)            # [S, DP]
    scores = (h @ h^T) * cw + cb      # [S, S]

Sharding over 8 NeuronCores: core c handles batch b = c//2 and the
i-half (rows) half = c%2 of the [S, S] score map. The same SPMD program
runs on every core; the i-half selection is realized on the host by
rolling X_b by -1024 rows for odd cores (so each core always computes
score rows 0:1024 of its local view), and un-rolling the output's j
axis on the host afterwards.

Per-core device program:
  - transpose X (PE, fp32 = exact) into XT [D, S] tiles
  - hT[p, s] = relu(W @ XT + pb) via PE matmuls in float32r
  - scores[i, j] = cw * sum_p hT[p,i] hT[p,j] + cb via PE f32r matmuls
  - stream out score tiles [128, 512]

float32r (TF32-like) matmuls run 4x faster than fp32 on the PE
(1 cycle/row vs 4) at ~1.6e-4 max relative error per 128-deep dot.
"""

import numpy as np

from concourse import bacc, masks, mybir, tile

P = 128
B, S, D = 4, 2048, 1024
DP = 256  # projection dim
IH = S // 2  # i-rows per core
NCORES = 8
KT = D // P  # 8 k-tiles over D
PT = DP // P  # 2 p-tiles over DP
SBLK = 512
NSB = S // SBLK  # 4 s-blocks

f32 = mybir.dt.float32
f32r = mybir.dt.float32r


def _build_nc():
    nc = bacc.Bacc()
    x = nc.declare_dram_parameter("x", [S, D], f32, isOutput=False)
    w = nc.declare_dram_parameter("w", [DP, D], f32, isOutput=False)
    pb = nc.declare_dram_parameter("pb", [DP], f32, isOutput=False)
    cwb = nc.declare_dram_parameter("cwb", [2], f32, isOutput=False)
    out = nc.declare_dram_parameter("out", [IH, S], f32, isOutput=True)

    with tile.TileContext(nc) as tc:
        _body(nc, tc, x, w, pb, cwb, out)
    nc.compile()
    return nc


def _body(nc, tc, x, w, pb, cwb, out):
    mult = mybir.AluOpType.mult
    add = mybir.AluOpType.add
    Relu = mybir.ActivationFunctionType.Relu
    Ident = mybir.ActivationFunctionType.Identity

    with (
        tc.tile_pool(name="const", bufs=1) as cpool,
        tc.tile_pool(name="xnat", bufs=2) as xpool,
        tc.tile_pool(name="xt", bufs=2) as xtpool,
        tc.tile_pool(name="outs", bufs=4) as opool,
        tc.tile_pool(name="tp", bufs=2, space="PSUM") as tp,
        tc.tile_pool(name="pj", bufs=2, space="PSUM") as pj,
        tc.tile_pool(name="pw", bufs=3, space="PSUM") as pw,
    ):
        # ---- constants / weights prep ----
        ident = cpool.tile([P, P], f32, tag="ident")
        masks.make_identity(nc, ident[:])

        w_nat = cpool.tile([P, PT, D], f32, tag="w_nat")
        nc.sync.dma_start(w_nat[:], w.ap().rearrange("(t p) d -> p t d", p=P))

        pb_t = cpool.tile([P, PT], f32, tag="pb_t")
        nc.sync.dma_start(pb_t[:], pb.ap().rearrange("(t p) -> p t", p=P))

        cwb_t = cpool.tile([P, 2], f32, tag="cwb_t")
        nc.sync.dma_start(cwb_t[:], cwb.ap().partition_broadcast(P))

        # WT[k][:, t*P:(t+1)*P] = W[t-tile, k-block]^T  (fp32 PE transpose, exact)
        wt = cpool.tile([P, KT, DP], f32r, tag="wt")
        for k in range(KT):
            tps = tp.tile([P, SBLK], f32, tag="tp")
            for t in range(PT):
                nc.tensor.transpose(
                    tps[:, t * P : (t + 1) * P],
                    w_nat[:, t, k * P : (k + 1) * P],
                    ident[:],
                )
            nc.vector.tensor_copy(wt[:, k, :], tps[:, 0:DP])

        # ---- projection: hT = relu(W @ XT + pb), computed per s-block ----
        ht = cpool.tile([P, PT, S], f32r, tag="ht")
        for sb in range(NSB):
            xn = xpool.tile([P, NSB, D], f32, tag="xn")
            nc.sync.dma_start(
                xn[:],
                x.ap()[sb * SBLK : (sb + 1) * SBLK, :].rearrange(
                    "(t p) d -> p t d", p=P
                ),
            )
            xt = xtpool.tile([P, KT, SBLK], f32r, tag="xt")
            for k in range(KT):
                tps = tp.tile([P, SBLK], f32, tag="tp")
                for t in range(SBLK // P):
                    nc.tensor.transpose(
                        tps[:, t * P : (t + 1) * P],
                        xn[:, t, k * P : (k + 1) * P],
                        ident[:],
                    )
                if k % 2 == 0:
                    nc.vector.tensor_copy(xt[:, k, :], tps[:])
                else:
                    nc.scalar.copy(xt[:, k, :], tps[:])

            for pt in range(PT):
                pjs = pj.tile([P, SBLK], f32, tag="pj")
                for k in range(KT):
                    nc.tensor.matmul(
                        pjs[:],
                        wt[:, k, pt * P : (pt + 1) * P],
                        xt[:, k, :],
                        start=(k == 0),
                        stop=(k == KT - 1),
                    )
                nc.scalar.activation(
                    ht[:, pt, sb * SBLK : (sb + 1) * SBLK],
                    pjs[:],
                    Relu,
                    bias=pb_t[:, pt : pt + 1],
                )

        # ---- pairwise: out[i, j] = cw * <h_i, h_j> + cb ----
        for it in range(IH // P):
            for jb in range(NSB):
                pws = pw.tile([P, SBLK], f32, tag="pw")
                for pt in range(PT):
                    nc.tensor.matmul(
                        pws[:],
                        ht[:, pt, it * P : (it + 1) * P],
                        ht[:, pt, jb * SBLK : (jb + 1) * SBLK],
                        start=(pt == 0),
                        stop=(pt == PT - 1),
                    )
                ot = opool.tile([P, SBLK], f32, tag="ot")
                if (it * NSB + jb) % 2 == 0:
                    nc.vector.tensor_scalar(
                        ot[:], pws[:], cwb_t[:, 0:1], cwb_t[:, 1:2], mult, add
                    )
                else:
                    nc.scalar.activation(
                        ot[:], pws[:], Ident,
                        bias=cwb_t[:, 1:2], scale=cwb_t[:, 0:1],
                    )
                nc.sync.dma_start(
                    out.ap()[it * P : (it + 1) * P, jb * SBLK : (jb + 1) * SBLK],
                    ot[:],
                )


_NC_CACHE = None


def _get_nc():
    global _NC_CACHE
    if _NC_CACHE is None:
        _NC_CACHE = _build_nc()
    return _NC_CACHE


def _make_in_maps(hidden_states, proj_w, proj_b, clf_w, clf_b):
    hs = np.ascontiguousarray(np.asarray(hidden_states, dtype=np.float32))
    wv = np.ascontiguousarray(np.asarray(proj_w, dtype=np.float32))
    pbv = np.ascontiguousarray(np.asarray(proj_b, dtype=np.float32).reshape(DP))
    cwbv = np.array(
        [np.asarray(clf_w).reshape(-1)[0], np.asarray(clf_b).reshape(-1)[0]],
        dtype=np.float32,
    )
    in_maps = []
    for c in range(NCORES):
        b, half = divmod(c, 2)
        xb = hs[b]
        if half:
            xb = np.ascontiguousarray(np.roll(xb, -IH, axis=0))
        in_maps.append({"x": xb, "w": wv, "pb": pbv, "cwb": cwbv})
    return in_maps


def _assemble(results):
    scores = np.empty((B, S, S), np.float32)
    for c in range(NCORES):
        b, half = divmod(c, 2)
        o = results[c]["out"]
        off = half * IH
        scores[b, off : off + IH, :] = np.roll(o, off, axis=1) if half else o
    return scores


def kernel(hidden_states, proj_w, proj_b, clf_w, clf_b):
    from concourse.bass_utils import run_bass_kernel_spmd

    nc = _get_nc()
    in_maps = _make_in_maps(hidden_states, proj_w, proj_b, clf_w, clf_b)
    res = run_bass_kernel_spmd(nc, in_maps, core_ids=list(range(NCORES)))
    return _assemble(res.results)


def run_traced(hidden_states, proj_w, proj_b, clf_w, clf_b):
    """Like kernel(), but also returns BassKernelResults with trace info."""
    from concourse.bass_utils import run_bass_kernel_spmd

    nc = _get_nc()
    in_maps = _make_in_maps(hidden_states, proj_w, proj_b, clf_w, clf_b)
    res = run_bass_kernel_spmd(
        nc, in_maps, core_ids=list(range(NCORES)), trace=True
    )
    return _assemble(res.results), res


# revision 4
# speedup vs baseline: 1.2663x; 1.2663x over previous
"""ContactMapHead Trainium2 kernel (v2: SYRK band + phase overlap).

Reference computation (per batch b):
    h = relu(X @ W^T + pb)            # [S, DP]
    scores = (h @ h^T) * cw + cb      # [S, S]  -- symmetric!

Sharding over 8 NeuronCores: core c handles batch b = c//2 with roll
offset off = (c%2)*1024 applied to X on the host. Each core computes
hT = relu(W @ XT + pb) for its full (rolled) batch, then emits the
circulant band of the symmetric score map: local tile rows i_t in 0..7
(tiles of 128), local tile cols j_t in i_t..i_t+8 (9 tiles of 128,
never wrapping since i_t <= 7). Across the two cores of a batch pair
(offsets 0 and 1024) plus host-side transpose mirroring, this covers
all 16x16 global tiles exactly (verified): global (r, c) with
(c - r) mod 16 in 0..8 is emitted directly, the rest by mirroring.

Device per-core program:
  - transpose X on the PE (fp32 = exact) into XT tiles, per s-block
  - hT[p, s-block] = relu(W @ XT + pb) via float32r matmuls
  - band rows: scores[i-tile, band] = cw * hT_i^T @ hT_band + cb
    (f32r matmuls, 1 LDWEIGHTS per (row, p-tile) amortized over 3
    512-boundary-aligned column chunks)
  - out strips [128, 1152] per row, one DMA each

float32r (TF32-like) matmuls run 4x faster than fp32 on the PE at
~1.6e-4 max relative error per 128-deep dot; PE fp32 transposes are
bit-exact.
"""

import numpy as np

from concourse import bacc, masks, mybir, tile

P = 128
B, S, D = 4, 2048, 1024
DP = 256  # projection dim
NCORES = 8
KT = D // P  # 8 k-tiles over D
PT = DP // P  # 2 p-tiles over DP
SBLK = 512
NSB = S // SBLK  # 4 s-blocks
NROW = 8  # local band rows (tiles of 128) per core
BANDW = 9 * P  # 1152 band columns per row

f32 = mybir.dt.float32
f32r = mybir.dt.float32r


def _row_segs(i_t):
    """512-aligned column segments of the band for local row i_t."""
    c0, c1 = i_t * P, i_t * P + BANDW
    segs = []
    c = c0
    while c < c1:
        nxt = min(c1, (c // SBLK + 1) * SBLK)
        segs.append((c, nxt))
        c = nxt
    return segs


def _build_nc():
    nc = bacc.Bacc()
    x = nc.declare_dram_parameter("x", [S, D], f32, isOutput=False)
    w = nc.declare_dram_parameter("w", [DP, D], f32, isOutput=False)
    pb = nc.declare_dram_parameter("pb", [DP], f32, isOutput=False)
    cwb = nc.declare_dram_parameter("cwb", [2], f32, isOutput=False)
    out = nc.declare_dram_parameter("out", [NROW, P, BANDW], f32, isOutput=True)

    with tile.TileContext(nc) as tc:
        _body(nc, tc, x, w, pb, cwb, out)
    nc.compile()
    return nc


def _body(nc, tc, x, w, pb, cwb, out):
    mult = mybir.AluOpType.mult
    add = mybir.AluOpType.add
    Relu = mybir.ActivationFunctionType.Relu
    Ident = mybir.ActivationFunctionType.Identity

    with (
        tc.tile_pool(name="const", bufs=1) as cpool,
        tc.tile_pool(name="xnat", bufs=2) as xpool,
        tc.tile_pool(name="xt", bufs=2) as xtpool,
        tc.tile_pool(name="orow", bufs=3) as opool,
        tc.tile_pool(name="tp", bufs=2, space="PSUM") as tp,
        tc.tile_pool(name="pj", bufs=2, space="PSUM") as pj,
        tc.tile_pool(name="pw", bufs=4, space="PSUM") as pw,
    ):
        # ---- constants / weights prep ----
        ident = cpool.tile([P, P], f32, tag="ident")
        masks.make_identity(nc, ident[:])

        w_nat = cpool.tile([P, PT, D], f32, tag="w_nat")
        nc.sync.dma_start(w_nat[:], w.ap().rearrange("(t p) d -> p t d", p=P))

        pb_t = cpool.tile([P, PT], f32, tag="pb_t")
        nc.sync.dma_start(pb_t[:], pb.ap().rearrange("(t p) -> p t", p=P))

        cwb_t = cpool.tile([P, 2], f32, tag="cwb_t")
        nc.sync.dma_start(cwb_t[:], cwb.ap().partition_broadcast(P))

        # WT[k][:, t*P:(t+1)*P] = W[t-tile, k-block]^T  (fp32 PE transpose)
        wt = cpool.tile([P, KT, DP], f32r, tag="wt")
        for k in range(KT):
            tps = tp.tile([P, SBLK], f32, tag="tp")
            for t in range(PT):
                nc.tensor.transpose(
                    tps[:, t * P : (t + 1) * P],
                    w_nat[:, t, k * P : (k + 1) * P],
                    ident[:],
                )
            nc.vector.tensor_copy(wt[:, k, :], tps[:, 0:DP])

        # per-s-block hT tiles so pairwise can start before projection ends
        ht = [
            cpool.tile([P, PT, SBLK], f32r, tag=f"ht{sb}", name=f"ht{sb}")
            for sb in range(NSB)
        ]

        def emit_pair_row(i_t):
            """Band row i_t: out[i_t] = cw * hT_i^T @ hT[band cols] + cb."""
            segs = _row_segs(i_t)
            psums = []
            for pt in range(PT):
                # one LDWEIGHTS per (row, pt): same stationary for all segs
                for si, (c0, c1) in enumerate(segs):
                    if pt == 0:
                        psums.append(
                            pw.tile([P, SBLK], f32, tag="pw", name="pw")
                        )
                    sb = c0 // SBLK
                    nc.tensor.matmul(
                        psums[si][:, 0 : c1 - c0],
                        ht[i_t // 4][:, pt, (i_t % 4) * P : (i_t % 4 + 1) * P],
                        ht[sb][:, pt, c0 - sb * SBLK : c1 - sb * SBLK],
                        start=(pt == 0),
                        stop=(pt == PT - 1),
                    )
            orow = opool.tile([P, BANDW], f32, tag="orow")
            base = i_t * P
            for si, (c0, c1) in enumerate(segs):
                args = (
                    orow[:, c0 - base : c1 - base],
                    psums[si][:, 0 : c1 - c0],
                )
                if (i_t * 3 + si) % 2 == 0:
                    nc.vector.tensor_scalar(
                        args[0], args[1], cwb_t[:, 0:1], cwb_t[:, 1:2], mult, add
                    )
                else:
                    nc.scalar.activation(
                        args[0], args[1], Ident,
                        bias=cwb_t[:, 1:2], scale=cwb_t[:, 0:1],
                    )
            nc.sync.dma_start(out.ap()[i_t], orow[:])

        # ---- main loop: stream s-blocks; band rows emitted when ready ----
        for sb in range(NSB):
            xn = xpool.tile([P, NSB, D], f32, tag="xn")
            xv = x.ap()[sb * SBLK : (sb + 1) * SBLK, :].rearrange(
                "(t p) d -> p t d", p=P
            )
            nc.sync.dma_start(xn[:, 0:2, :], xv[:, 0:2, :])
            nc.sync.dma_start(xn[:, 2:4, :], xv[:, 2:4, :])

            xt = xtpool.tile([P, KT, SBLK], f32r, tag="xt")
            for k in range(KT):
                tps = tp.tile([P, SBLK], f32, tag="tp")
                for t in range(SBLK // P):
                    nc.tensor.transpose(
                        tps[:, t * P : (t + 1) * P],
                        xn[:, t, k * P : (k + 1) * P],
                        ident[:],
                    )
                if k % 2 == 0:
                    nc.vector.tensor_copy(xt[:, k, :], tps[:])
                else:
                    nc.scalar.copy(xt[:, k, :], tps[:])

            for pt in range(PT):
                pjs = pj.tile([P, SBLK], f32, tag="pj")
                for k in range(KT):
                    nc.tensor.matmul(
                        pjs[:],
                        wt[:, k, pt * P : (pt + 1) * P],
                        xt[:, k, :],
                        start=(k == 0),
                        stop=(k == KT - 1),
                    )
                nc.scalar.activation(
                    ht[sb][:, pt, :],
                    pjs[:],
                    Relu,
                    bias=pb_t[:, pt : pt + 1],
                )

            # band rows whose dependencies just completed:
            # rows 0..3 need s-blocks 0..2; rows 4..7 need all 4.
            if sb == 2:
                for i_t in range(4):
                    emit_pair_row(i_t)
            elif sb == 3:
                for i_t in range(4, NROW):
                    emit_pair_row(i_t)


_NC_CACHE = None


def _get_nc():
    global _NC_CACHE
    if _NC_CACHE is None:
        _NC_CACHE = _build_nc()
    return _NC_CACHE


def _make_in_maps(hidden_states, proj_w, proj_b, clf_w, clf_b):
    hs = np.ascontiguousarray(np.asarray(hidden_states, dtype=np.float32))
    wv = np.ascontiguousarray(np.asarray(proj_w, dtype=np.float32))
    pbv = np.ascontiguousarray(np.asarray(proj_b, dtype=np.float32).reshape(DP))
    cwbv = np.array(
        [np.asarray(clf_w).reshape(-1)[0], np.asarray(clf_b).reshape(-1)[0]],
        dtype=np.float32,
    )
    in_maps = []
    for c in range(NCORES):
        b, half = divmod(c, 2)
        xb = hs[b]
        if half:
            xb = np.ascontiguousarray(np.roll(xb, -S // 2, axis=0))
        in_maps.append({"x": xb, "w": wv, "pb": pbv, "cwb": cwbv})
    return in_maps


def _assemble(results):
    scores = np.empty((B, S, S), np.float32)
    for c in range(NCORES):
        b, half = divmod(c, 2)
        o = results[c]["out"]  # [NROW, P, BANDW]
        for i_t in range(NROW):
            gi = i_t + NROW * half
            strip = o[i_t]
            for lj in range(i_t, i_t + 9):
                gj = (lj + NROW * half) % 16
                V = strip[:, (lj - i_t) * P : (lj - i_t + 1) * P]
                scores[b, gi * P : (gi + 1) * P, gj * P : (gj + 1) * P] = V
                if gj != gi:
                    scores[b, gj * P : (gj + 1) * P, gi * P : (gi + 1) * P] = V.T
    return scores


def kernel(hidden_states, proj_w, proj_b, clf_w, clf_b):
    from concourse.bass_utils import run_bass_kernel_spmd

    nc = _get_nc()
    in_maps = _make_in_maps(hidden_states, proj_w, proj_b, clf_w, clf_b)
    res = run_bass_kernel_spmd(nc, in_maps, core_ids=list(range(NCORES)))
    return _assemble(res.results)


def run_traced(hidden_states, proj_w, proj_b, clf_w, clf_b):
    """Like kernel(), but also returns BassKernelResults with trace info."""
    from concourse.bass_utils import run_bass_kernel_spmd

    nc = _get_nc()
    in_maps = _make_in_maps(hidden_states, proj_w, proj_b, clf_w, clf_b)
    res = run_bass_kernel_spmd(
        nc, in_maps, core_ids=list(range(NCORES)), trace=True
    )
    return _assemble(res.results), res
